# revision 6
# baseline (speedup 1.0000x reference)
"""Trainium2 Bass kernel for nn_Classifier2_54022098649409 (GNN message passing).

Distribution (8 cores):
  L1: node-equal slices — dilated convs as shifted matmuls + per-node transform
      -> gather table T1 [N, 192] (cols 0:162 = h01@W11 | h02@W12 | h03@W13).
  L2/L3/L4: graph-boundary node slices (32 graphs/core); edges owned by dst core,
      grouped by 128-node dst block. Per edge chunk (128 edges):
      dma_gather rows from the table, build one-hot P'[e,r]=(iota==dstnorm)*ew on
      DVE, scatter-add via PE matmul P'^T @ msgs accumulated in PSUM per block.
  L2 -> ac_h1 -> T2 = ac_h1@W2 (pair-packed table [N/2, 64]).
  L3 -> ac_h2 (-> T3 pair-packed).
  L4 -> S=scatter(T3); ac_h3/4=relu(S@[W3|W4]+b); readout segment-sum by graph id
      via one-hot matmul, folded linear head W' = Wc1@Wc2.
Host does sharding/index prep + pure data-movement reassembly between launches.
"""

import numpy as np

import concourse.bacc as bacc
import concourse.bass as bass
import concourse.mybir as mybir
import concourse.tile as tile
from concourse.bass_utils import run_bass_kernel_spmd
from concourse.masks import make_identity

F32 = mybir.dt.float32
I16 = mybir.dt.int16
I32 = mybir.dt.int32
AOT = mybir.AluOpType

N, E, G, M = 51200, 819200, 256, 8
GPC = G // M          # graphs per core
SL = N // M           # L1 equal slice
HALF = N // 2         # 25600 (table half rows / pair rows)
F1, P1 = 162, 192     # L1 table width / padded width
FP = 64               # pair-packed row width (25 | pad | 25 | pad)

CORES = list(range(M))

# collected by test harness (exec_time_ns per launch when BASS_TRACE=1)
LAST_EXEC_NS = []

_prog_cache = {}


# --------------------------------------------------------------------------
# host-side prep
# --------------------------------------------------------------------------

def _host_prep(inputs):
    x = np.ascontiguousarray(inputs["x"], np.float32)
    src = np.asarray(inputs["src"]).astype(np.int64)
    dst = np.asarray(inputs["dst"]).astype(np.int64)
    gid = np.asarray(inputs["graph_ids"]).astype(np.int64)
    ew = np.ascontiguousarray(inputs["edge_weight"], np.float32)

    p = {}
    p["xT"] = np.ascontiguousarray(x.T)  # [128, N]

    # conv weights [128, 9*96]: block (d*3+k) = conv{d}_w[:, :, k].T
    cw = np.zeros((128, 9 * 96), np.float32)
    cb = np.zeros((96, 3), np.float32)
    for d in range(3):
        w = np.asarray(inputs[f"conv{d+1}_w"], np.float32)  # [96,128,3]
        for k in range(3):
            cw[:, (d * 3 + k) * 96:(d * 3 + k + 1) * 96] = w[:, :, k].T
        cb[:, d] = np.asarray(inputs[f"conv{d+1}_b"], np.float32)
    p["cw"], p["cb"] = cw, cb
    p["w1"] = np.ascontiguousarray(
        np.concatenate([np.asarray(inputs[w], np.float32)
                        for w in ("W11", "W12", "W13")], axis=1))  # [96,162]

    # graph-boundary slices
    starts = np.searchsorted(gid, np.arange(0, G, GPC))
    ends = np.concatenate([starts[1:], [N]])
    NB = int(np.max(np.ceil((ends - starts) / 128)).astype(int))
    p["starts"], p["ends"], p["NB"] = starts, ends, NB

    core_of_edge = np.searchsorted(starts, dst, side="right") - 1

    # ---- edge schedules ----
    def schedule(nhalf, half_arr, idx_arr):
        """Group edges per core by (dst block, half); pad runs to 128.
        Returns per-core dict of arrays + CPBH."""
        percore = []
        cnts = np.zeros((M, NB, nhalf), np.int64)
        for k in range(M):
            sel = np.where(core_of_edge == k)[0]
            dl = dst[sel] - starts[k]
            blk = dl // 128
            hh = half_arr[sel]
            order = np.lexsort((hh, blk))
            sel, dl, blk, hh = sel[order], dl[order], blk[order], hh[order]
            # run boundaries
            for b in range(NB):
                m0 = blk == b
                for h in range(nhalf):
                    cnts[k, b, h] = np.sum(m0 & (hh == h))
            percore.append((sel, dl))
        CPBH = int(np.max(np.ceil(cnts / 128)))
        out = []
        for k in range(M):
            sel, dl = percore[k]
            nch = NB * nhalf * CPBH
            idx = np.zeros(nch * 128, np.int16)
            dstn = np.zeros(nch * 128, np.float32)
            ewt = np.zeros(nch * 128, np.float32)
            par = np.zeros(nch * 128, np.float32)
            pos = 0
            run = 0
            for b in range(NB):
                for h in range(nhalf):
                    c = int(cnts[k, b, h])
                    s = slice(pos, pos + c)
                    o = run * 128 * CPBH
                    idx[o:o + c] = idx_arr[sel[s]]
                    dstn[o:o + c] = (dl[s] - b * 128).astype(np.float32)
                    ewt[o:o + c] = ew[sel[s]]
                    par[o:o + c] = ((src[sel[s]] % 128) // 64).astype(np.float32)
                    pos += c
                    run += 1
            out.append(dict(idx=idx, dstn=dstn, ewt=ewt, par=par))
        return out, CPBH

    sched2, CH2 = schedule(2, src // HALF, (src % HALF).astype(np.int16))
    pair = ((src // 128) * 64 + (src % 64)).astype(np.int16)
    sched3, CH3 = schedule(1, np.zeros(E, np.int64), pair)
    p["CH2"], p["CH3"] = CH2, CH3

    def finish(sched, CPBH, nhalf):
        out = []
        for k in range(M):
            s = sched[k]
            nch = NB * nhalf * CPBH
            d = dict(
                idx=np.ascontiguousarray(
                    np.tile(s["idx"].reshape(-1, 16).T, (8, 1))),  # [128, nch*8]
                dstn=np.ascontiguousarray(s["dstn"].reshape(nch, 128).T),  # [128, nch]
                ewt=np.ascontiguousarray(s["ewt"].reshape(nch, 128).T),
                par=np.ascontiguousarray(s["par"].reshape(nch, 128).T),
            )
            d["inv"] = np.ascontiguousarray(1.0 - d["par"])
            out.append(d)
        return out
    p["e2"] = finish(sched2, CH2, 2)
    p["e3"] = finish(sched3, CH3, 1)

    # small weights
    p["bia"] = np.ascontiguousarray(np.tile(np.concatenate(
        [np.asarray(inputs[b], np.float32) for b in ("b11", "b12", "b13")]
    )[None, :], (128, 1)))                                      # [128,162]
    p["w2"] = np.ascontiguousarray(np.asarray(inputs["W2"], np.float32))   # [54,25]
    p["b2b"] = np.ascontiguousarray(
        np.tile(np.asarray(inputs["b2"], np.float32)[None, :], (128, 1)))  # [128,25]
    p["w34"] = np.ascontiguousarray(np.concatenate(
        [np.asarray(inputs["W3"], np.float32),
         np.asarray(inputs["W4"], np.float32)], axis=1))        # [25,24]
    p["b34b"] = np.ascontiguousarray(np.tile(np.concatenate(
        [np.asarray(inputs["b3"], np.float32),
         np.asarray(inputs["b4"], np.float32)])[None, :], (128, 1)))  # [128,24]
    wp = (np.asarray(inputs["Wc1"], np.float32)
          @ np.asarray(inputs["Wc2"], np.float32)).astype(np.float32)  # [49,5]
    bp = (np.asarray(inputs["bc1"], np.float32)
          @ np.asarray(inputs["Wc2"], np.float32)
          + np.asarray(inputs["bc2"], np.float32)).astype(np.float32)  # [5]
    p["wp"] = np.ascontiguousarray(wp)
    p["bpb"] = np.ascontiguousarray(np.tile(bp[None, :], (GPC, 1)))  # [32,5]

    # per-core graph-id-norm per node tile [128, NB]
    gidn = []
    for k in range(M):
        g = np.full(NB * 128, -1.0, np.float32)
        n = ends[k] - starts[k]
        g[:n] = (gid[starts[k]:ends[k]] - k * GPC).astype(np.float32)
        gidn.append(np.ascontiguousarray(g.reshape(NB, 128).T))  # [128, NB]
    p["gidn"] = gidn
    return p


def _pack_pairs(tbl):
    """tbl [N, 25] -> [N/2, 64]: row b*64+j = [t[b*128+j] |pad| t[b*128+64+j] |pad]"""
    t = tbl.reshape(N // 128, 2, 64, 25)
    out = np.zeros((N // 2, FP), np.float32)
    o = out.reshape(N // 128, 64, FP)
    o[:, :, 0:25] = t[:, 0]
    o[:, :, 32:57] = t[:, 1]
    return out


# --------------------------------------------------------------------------
# bass program builders
# --------------------------------------------------------------------------

def _new_nc():
    return bacc.Bacc("TRN2", target_bir_lowering=False, debug=False,
                     enable_asserts=False, num_devices=M, num_swdge_queues=4)


def _build_l1():
    nc = _new_nc()
    xs = nc.dram_tensor("xs", [128, SL + 6], F32, kind="ExternalInput").ap()
    cw = nc.dram_tensor("cw", [128, 9 * 96], F32, kind="ExternalInput").ap()
    cb = nc.dram_tensor("cb", [96, 3], F32, kind="ExternalInput").ap()
    w1 = nc.dram_tensor("w1", [96, F1], F32, kind="ExternalInput").ap()
    t1 = nc.dram_tensor("t1", [SL, P1], F32, kind="ExternalOutput").ap()

    with tile.TileContext(nc) as tc:
        with (
            tc.tile_pool(name="consts", bufs=1) as consts,
            tc.tile_pool(name="work", bufs=3) as sb,
            tc.tile_pool(name="psc", bufs=4, space="PSUM") as ps_c,
            tc.tile_pool(name="pst", bufs=2, space="PSUM") as ps_t,
        ):
            xs_sb = consts.tile([128, SL + 6], F32)
            nc.sync.dma_start(out=xs_sb[:], in_=xs[:])
            cw_sb = consts.tile([128, 9 * 96], F32)
            nc.sync.dma_start(out=cw_sb[:], in_=cw[:])
            cb_sb = consts.tile([96, 3], F32)
            nc.sync.dma_start(out=cb_sb[:], in_=cb[:])
            w1_sb = consts.tile([96, F1], F32)
            nc.sync.dma_start(out=w1_sb[:], in_=w1[:])

            for t in range(SL // 128):
                c_sb = []
                for d in range(3):
                    pc = ps_c.tile([96, 128], F32, tag="pc")
                    for k in range(3):
                        off = 3 + t * 128 + (k - 1) * (d + 1)
                        nc.tensor.matmul(
                            out=pc[:],
                            lhsT=cw_sb[:, (d * 3 + k) * 96:(d * 3 + k + 1) * 96],
                            rhs=xs_sb[:, off:off + 128],
                            start=(k == 0), stop=(k == 2))
                    cs = sb.tile([96, 128], F32, tag=f"c{d}")
                    nc.vector.tensor_scalar(
                        out=cs[:], in0=pc[:], scalar1=cb_sb[:, d:d + 1],
                        scalar2=None, op0=AOT.add)
                    c_sb.append(cs)
                h_sb = []
                for i in range(3):
                    r = sb.tile([96, 128], F32, tag=f"h{i}")
                    nc.vector.tensor_scalar_max(out=r[:], in0=c_sb[i][:], scalar1=0.0)
                    nc.vector.tensor_tensor(
                        out=r[:], in0=r[:], in1=c_sb[(i + 1) % 3][:], op=AOT.add)
                    h_sb.append(r)
                o_sb = sb.tile([128, F1], F32, tag="o")
                for i in range(3):
                    pt = ps_t.tile([128, 54], F32, tag="pt")
                    nc.tensor.matmul(out=pt[:], lhsT=h_sb[i][:],
                                     rhs=w1_sb[:, i * 54:(i + 1) * 54],
                                     start=True, stop=True)
                    nc.vector.tensor_copy(out=o_sb[:, i * 54:(i + 1) * 54], in_=pt[:])
                nc.sync.dma_start(out=t1[t * 128:(t + 1) * 128, 0:F1], in_=o_sb[:])
    nc.compile()
    return nc


def _build_scatter_common(nc, tc, consts, NB, CH, nhalf):
    """Declare edge-schedule inputs + iota/identity and load them to SBUF."""
    nch = NB * nhalf * CH
    eidx = nc.dram_tensor("eidx", [128, nch * 8], I16, kind="ExternalInput").ap()
    edst = nc.dram_tensor("edst", [128, nch], F32, kind="ExternalInput").ap()
    eewt = nc.dram_tensor("eewt", [128, nch], F32, kind="ExternalInput").ap()

    eidx_sb = consts.tile([128, nch * 8], I16)
    nc.sync.dma_start(out=eidx_sb[:], in_=eidx[:])
    edst_sb = consts.tile([128, nch], F32)
    nc.sync.dma_start(out=edst_sb[:], in_=edst[:])
    eewt_sb = consts.tile([128, nch], F32)
    nc.sync.dma_start(out=eewt_sb[:], in_=eewt[:])

    iota_i = consts.tile([128, 128], I32)
    nc.gpsimd.iota(out=iota_i[:], pattern=[[1, 128]], base=0, channel_multiplier=0)
    iota_f = consts.tile([128, 128], F32)
    nc.vector.tensor_copy(out=iota_f[:], in_=iota_i[:])
    ident = consts.tile([128, 128], F32)
    make_identity(nc, ident[:])
    return eidx_sb, edst_sb, eewt_sb, iota_f, ident


def _build_l2(NB, CH):
    nc = _new_nc()
    t1lo = nc.dram_tensor("t1lo", [HALF, P1], F32, kind="ExternalInput").ap()
    t1hi = nc.dram_tensor("t1hi", [HALF, P1], F32, kind="ExternalInput").ap()
    bia = nc.dram_tensor("bia", [128, F1], F32, kind="ExternalInput").ap()
    w2 = nc.dram_tensor("w2", [54, 25], F32, kind="ExternalInput").ap()
    t2 = nc.dram_tensor("t2", [NB * 128, 25], F32, kind="ExternalOutput").ap()

    with tile.TileContext(nc) as tc:
        with (
            tc.tile_pool(name="consts", bufs=1) as consts,
            tc.tile_pool(name="gath", bufs=3) as gpool,
            tc.tile_pool(name="pm", bufs=4) as pmpool,
            tc.tile_pool(name="misc", bufs=3) as spool,
            tc.tile_pool(name="pss", bufs=2, space="PSUM") as ps_s,
            tc.tile_pool(name="pstr", bufs=2, space="PSUM") as ps_tr,
            tc.tile_pool(name="pso", bufs=2, space="PSUM") as ps_o,
        ):
            eidx_sb, edst_sb, eewt_sb, iota_f, ident = _build_scatter_common(
                nc, tc, consts, NB, CH, 2)
            bia_sb = consts.tile([128, F1], F32)
            nc.sync.dma_start(out=bia_sb[:], in_=bia[:])
            w2_sb = consts.tile([54, 25], F32)
            nc.sync.dma_start(out=w2_sb[:], in_=w2[:])

            for b in range(NB):
                ps = ps_s.tile([128, F1], F32, tag="s")
                for h in range(2):
                    call = b * 2 + h
                    g = gpool.tile([128, CH * P1], F32, tag="g")
                    nc.gpsimd.dma_gather(
                        out_ap=g[:].rearrange("p (c e) -> p c e", e=P1),
                        in_ap=(t1lo if h == 0 else t1hi)[:, :],
                        idxs_ap=eidx_sb[:, call * CH * 8:(call + 1) * CH * 8],
                        num_idxs=CH * 128,
                        num_idxs_reg=CH * 128,
                        elem_size=P1,
                        queue_num=call % 4,
                        single_packet=False,
                    )
                    for j in range(CH):
                        c = call * CH + j
                        pm = pmpool.tile([128, 128], F32, tag="pm")
                        nc.vector.tensor_scalar(
                            out=pm[:], in0=iota_f[:],
                            scalar1=edst_sb[:, c:c + 1],
                            scalar2=eewt_sb[:, c:c + 1],
                            op0=AOT.is_equal, op1=AOT.mult)
                        nc.tensor.matmul(
                            out=ps[:], lhsT=pm[:],
                            rhs=g[:, j * P1:j * P1 + F1],
                            start=(h == 0 and j == 0),
                            stop=(h == 1 and j == CH - 1))
                # ac_h1 = sum_i relu(S_i + b_i)
                tacc = spool.tile([128, F1], F32, tag="t")
                nc.vector.tensor_tensor(out=tacc[:], in0=ps[:], in1=bia_sb[:],
                                        op=AOT.add)
                nc.vector.tensor_scalar_max(out=tacc[:], in0=tacc[:], scalar1=0.0)
                ac = spool.tile([128, 54], F32, tag="ac")
                nc.vector.tensor_tensor(out=ac[:], in0=tacc[:, 0:54],
                                        in1=tacc[:, 54:108], op=AOT.add)
                nc.vector.tensor_tensor(out=ac[:], in0=ac[:],
                                        in1=tacc[:, 108:F1], op=AOT.add)
                ptr = ps_tr.tile([54, 128], F32, tag="tr")
                nc.tensor.transpose(out=ptr[:], in_=ac[:], identity=ident[:])
                acT = spool.tile([54, 128], F32, tag="acT")
                nc.vector.tensor_copy(out=acT[:], in_=ptr[:])
                po = ps_o.tile([128, 25], F32, tag="po")
                nc.tensor.matmul(out=po[:], lhsT=acT[:], rhs=w2_sb[:],
                                 start=True, stop=True)
                ot = spool.tile([128, 25], F32, tag="ot")
                nc.vector.tensor_copy(out=ot[:], in_=po[:])
                nc.sync.dma_start(out=t2[b * 128:(b + 1) * 128, :], in_=ot[:])
    nc.compile()
    return nc


def _scatter_pass_small(nc, tc, pools, b, CH, tbl, eidx_sb, edst_sb, eewt_sb,
                        epar_sb, einv_sb, iota_f):
    """Gather pair-packed rows + parity select + one-hot matmul scatter.
    Returns psum tile [128, 25] holding S for block b."""
    gpool, pmpool, mpool, ps_s = pools
    ps = ps_s.tile([128, 25], F32, tag="s")
    g = gpool.tile([128, CH * FP], F32, tag="g")
    nc.gpsimd.dma_gather(
        out_ap=g[:].rearrange("p (c e) -> p c e", e=FP),
        in_ap=tbl[:, :],
        idxs_ap=eidx_sb[:, b * CH * 8:(b + 1) * CH * 8],
        num_idxs=CH * 128,
        num_idxs_reg=CH * 128,
        elem_size=FP,
        queue_num=b % 4,
        single_packet=False,
    )
    for j in range(CH):
        c = b * CH + j
        m = mpool.tile([128, 25], F32, tag="m")
        nc.vector.tensor_scalar(
            out=m[:], in0=g[:, j * FP:j * FP + 25],
            scalar1=einv_sb[:, c:c + 1], scalar2=None, op0=AOT.mult)
        m2 = mpool.tile([128, 25], F32, tag="m2")
        nc.vector.tensor_scalar(
            out=m2[:], in0=g[:, j * FP + 32:j * FP + 57],
            scalar1=epar_sb[:, c:c + 1], scalar2=None, op0=AOT.mult)
        nc.vector.tensor_tensor(out=m[:], in0=m[:], in1=m2[:], op=AOT.add)
        pm = pmpool.tile([128, 128], F32, tag="pm")
        nc.vector.tensor_scalar(
            out=pm[:], in0=iota_f[:],
            scalar1=edst_sb[:, c:c + 1], scalar2=eewt_sb[:, c:c + 1],
            op0=AOT.is_equal, op1=AOT.mult)
        nc.tensor.matmul(out=ps[:], lhsT=pm[:], rhs=m[:],
                         start=(j == 0), stop=(j == CH - 1))
    return ps


def _declare_parity(nc, consts, NB, CH):
    nch = NB * CH
    epar = nc.dram_tensor("epar", [128, nch], F32, kind="ExternalInput").ap()
    einv = nc.dram_tensor("einv", [128, nch], F32, kind="ExternalInput").ap()
    epar_sb = consts.tile([128, nch], F32)
    nc.sync.dma_start(out=epar_sb[:], in_=epar[:])
    einv_sb = consts.tile([128, nch], F32)
    nc.sync.dma_start(out=einv_sb[:], in_=einv[:])
    return epar_sb, einv_sb


def _build_l3(NB, CH):
    nc = _new_nc()
    t2p = nc.dram_tensor("t2p", [HALF, FP], F32, kind="ExternalInput").ap()
    b2b = nc.dram_tensor("b2b", [128, 25], F32, kind="ExternalInput").ap()
    ach2 = nc.dram_tensor("ach2", [NB * 128, 25], F32, kind="ExternalOutput").ap()

    with tile.TileContext(nc) as tc:
        with (
            tc.tile_pool(name="consts", bufs=1) as consts,
            tc.tile_pool(name="gath", bufs=3) as gpool,
            tc.tile_pool(name="pm", bufs=4) as pmpool,
            tc.tile_pool(name="msel", bufs=4) as mpool,
            tc.tile_pool(name="misc", bufs=3) as spool,
            tc.tile_pool(name="pss", bufs=2, space="PSUM") as ps_s,
        ):
            eidx_sb, edst_sb, eewt_sb, iota_f, ident = _build_scatter_common(
                nc, tc, consts, NB, CH, 1)
            epar_sb, einv_sb = _declare_parity(nc, consts, NB, CH)
            b2b_sb = consts.tile([128, 25], F32)
            nc.sync.dma_start(out=b2b_sb[:], in_=b2b[:])

            pools = (gpool, pmpool, mpool, ps_s)
            for b in range(NB):
                ps = _scatter_pass_small(nc, tc, pools, b, CH, t2p, eidx_sb,
                                         edst_sb, eewt_sb, epar_sb, einv_sb,
                                         iota_f)
                ac = spool.tile([128, 25], F32, tag="ac")
                nc.vector.tensor_tensor(out=ac[:], in0=ps[:], in1=b2b_sb[:],
                                        op=AOT.add)
                nc.vector.tensor_scalar_max(out=ac[:], in0=ac[:], scalar1=0.0)
                nc.sync.dma_start(out=ach2[b * 128:(b + 1) * 128, :], in_=ac[:])
    nc.compile()
    return nc


def _build_l4(NB, CH):
    nc = _new_nc()
    t3p = nc.dram_tensor("t3p", [HALF, FP], F32, kind="ExternalInput").ap()
    ach2k = nc.dram_tensor("ach2k", [NB * 128, 25], F32, kind="ExternalInput").ap()
    w34 = nc.dram_tensor("w34", [25, 24], F32, kind="ExternalInput").ap()
    b34b = nc.dram_tensor("b34b", [128, 24], F32, kind="ExternalInput").ap()
    gidn = nc.dram_tensor("gidn", [128, NB], F32, kind="ExternalInput").ap()
    wp = nc.dram_tensor("wp", [49, 5], F32, kind="ExternalInput").ap()
    bpb = nc.dram_tensor("bpb", [GPC, 5], F32, kind="ExternalInput").ap()
    outk = nc.dram_tensor("outk", [GPC, 5], F32, kind="ExternalOutput").ap()

    with tile.TileContext(nc) as tc:
        with (
            tc.tile_pool(name="consts", bufs=1) as consts,
            tc.tile_pool(name="gath", bufs=3) as gpool,
            tc.tile_pool(name="pm", bufs=4) as pmpool,
            tc.tile_pool(name="msel", bufs=4) as mpool,
            tc.tile_pool(name="misc", bufs=3) as spool,
            tc.tile_pool(name="pss", bufs=2, space="PSUM") as ps_s,
            tc.tile_pool(name="pstr", bufs=2, space="PSUM") as ps_tr,
            tc.tile_pool(name="psh", bufs=2, space="PSUM") as ps_h,
            tc.tile_pool(name="psr", bufs=1, space="PSUM") as ps_r,
            # PSUM budget: pss 2 + pstr 2 (shared tag) + psh 2 (shared tag)
            # + psr 1 = 7 banks
        ):
            eidx_sb, edst_sb, eewt_sb, iota_f, ident = _build_scatter_common(
                nc, tc, consts, NB, CH, 1)
            epar_sb, einv_sb = _declare_parity(nc, consts, NB, CH)
            w34_sb = consts.tile([25, 24], F32)
            nc.sync.dma_start(out=w34_sb[:], in_=w34[:])
            b34b_sb = consts.tile([128, 24], F32)
            nc.sync.dma_start(out=b34b_sb[:], in_=b34b[:])
            gidn_sb = consts.tile([128, NB], F32)
            nc.sync.dma_start(out=gidn_sb[:], in_=gidn[:])
            wp_sb = consts.tile([49, 5], F32)
            nc.sync.dma_start(out=wp_sb[:], in_=wp[:])
            bpb_sb = consts.tile([GPC, 5], F32)
            nc.sync.dma_start(out=bpb_sb[:], in_=bpb[:])

            pr = ps_r.tile([GPC, 49], F32, tag="r")
            pools = (gpool, pmpool, mpool, ps_s)
            for b in range(NB):
                ps = _scatter_pass_small(nc, tc, pools, b, CH, t3p, eidx_sb,
                                         edst_sb, eewt_sb, epar_sb, einv_sb,
                                         iota_f)
                s_sb = spool.tile([128, 25], F32, tag="sb")
                nc.vector.tensor_copy(out=s_sb[:], in_=ps[:])
                ptr = ps_tr.tile([25, 128], F32, tag="tr")
                nc.tensor.transpose(out=ptr[:], in_=s_sb[:], identity=ident[:])
                st = spool.tile([25, 128], F32, tag="st")
                nc.vector.tensor_copy(out=st[:], in_=ptr[:])
                ph = ps_h.tile([128, 24], F32, tag="h")
                nc.tensor.matmul(out=ph[:], lhsT=st[:], rhs=w34_sb[:],
                                 start=True, stop=True)
                feat = spool.tile([128, 49], F32, tag="feat")
                nc.sync.dma_start(out=feat[:, 0:25],
                                  in_=ach2k[b * 128:(b + 1) * 128, :])
                nc.vector.tensor_tensor(out=feat[:, 25:49], in0=ph[:],
                                        in1=b34b_sb[:], op=AOT.add)
                nc.vector.tensor_scalar_max(out=feat[:, 25:49],
                                            in0=feat[:, 25:49], scalar1=0.0)
                pg = pmpool.tile([128, GPC], F32, tag="pg")
                nc.vector.tensor_scalar(
                    out=pg[:], in0=iota_f[:, 0:GPC],
                    scalar1=gidn_sb[:, b:b + 1], scalar2=None, op0=AOT.is_equal)
                nc.tensor.matmul(out=pr[:], lhsT=pg[:], rhs=feat[:],
                                 start=(b == 0), stop=(b == NB - 1))
            r_sb = spool.tile([GPC, 49], F32, tag="rsb")
            nc.vector.tensor_copy(out=r_sb[:], in_=pr[:])
            prt = ps_tr.tile([49, GPC], F32, tag="tr")
            nc.tensor.transpose(out=prt[:], in_=r_sb[:],
                                identity=ident[0:GPC, 0:GPC])
            rt_sb = spool.tile([49, GPC], F32, tag="rts")
            nc.vector.tensor_copy(out=rt_sb[:], in_=prt[:])
            po = ps_h.tile([GPC, 5], F32, tag="h")
            nc.tensor.matmul(out=po[:], lhsT=rt_sb[:], rhs=wp_sb[:],
                             start=True, stop=True)
            o_sb = spool.tile([GPC, 5], F32, tag="ob")
            nc.vector.tensor_tensor(out=o_sb[:], in0=po[:], in1=bpb_sb[:],
                                    op=AOT.add)
            nc.sync.dma_start(out=outk[:], in_=o_sb[:])
    nc.compile()
    return nc


# --------------------------------------------------------------------------
# run helpers
# --------------------------------------------------------------------------

def _run(nc, in_maps):
    res = run_bass_kernel_spmd(nc, in_maps, core_ids=CORES)
    if res.exec_time_ns is not None:
        LAST_EXEC_NS.append(res.exec_time_ns)
    return res.results


def _get_prog(key, builder, *args):
    if key not in _prog_cache:
        _prog_cache[key] = builder(*args)
    return _prog_cache[key]


def kernel(**inputs):
    LAST_EXEC_NS.clear()
    p = _host_prep(inputs)
    NB, CH2, CH3 = p["NB"], p["CH2"], p["CH3"]
    starts, ends = p["starts"], p["ends"]

    # ---------------- L1 ----------------
    nc1 = _get_prog("l1", _build_l1)
    xTp = np.zeros((128, N + 6), np.float32)
    xTp[:, 3:3 + N] = p["xT"]
    in1 = [{
        "xs": np.ascontiguousarray(xTp[:, k * SL:k * SL + SL + 6]),
        "cw": p["cw"], "cb": p["cb"], "w1": p["w1"],
    } for k in range(M)]
    r1 = _run(nc1, in1)
    T1 = np.concatenate([r1[k]["t1"] for k in range(M)], axis=0)
    T1[:, F1:] = 0.0
    t1lo = np.ascontiguousarray(T1[:HALF])
    t1hi = np.ascontiguousarray(T1[HALF:])

    # ---------------- L2 ----------------
    nc2 = _get_prog(("l2", NB, CH2), _build_l2, NB, CH2)
    in2 = [{
        "t1lo": t1lo, "t1hi": t1hi,
        "eidx": p["e2"][k]["idx"], "edst": p["e2"][k]["dstn"],
        "eewt": p["e2"][k]["ewt"],
        "bia": p["bia"], "w2": p["w2"],
    } for k in range(M)]
    r2 = _run(nc2, in2)
    T2g = np.zeros((N, 25), np.float32)
    for k in range(M):
        n = ends[k] - starts[k]
        T2g[starts[k]:ends[k]] = r2[k]["t2"][:n]
    t2p = _pack_pairs(T2g)

    # ---------------- L3 ----------------
    nc3 = _get_prog(("l3", NB, CH3), _build_l3, NB, CH3)
    in3 = [{
        "t2p": t2p,
        "eidx": p["e3"][k]["idx"], "edst": p["e3"][k]["dstn"],
        "eewt": p["e3"][k]["ewt"], "epar": p["e3"][k]["par"],
        "einv": p["e3"][k]["inv"],
        "b2b": p["b2b"],
    } for k in range(M)]
    r3 = _run(nc3, in3)
    ach2 = [r3[k]["ach2"] for k in range(M)]
    T3g = np.zeros((N, 25), np.float32)
    for k in range(M):
        n = ends[k] - starts[k]
        T3g[starts[k]:ends[k]] = ach2[k][:n]
    t3p = _pack_pairs(T3g)

    # ---------------- L4 ----------------
    nc4 = _get_prog(("l4", NB, CH3), _build_l4, NB, CH3)
    in4 = [{
        "t3p": t3p,
        "eidx": p["e3"][k]["idx"], "edst": p["e3"][k]["dstn"],
        "eewt": p["e3"][k]["ewt"], "epar": p["e3"][k]["par"],
        "einv": p["e3"][k]["inv"],
        "ach2k": np.ascontiguousarray(ach2[k]),
        "w34": p["w34"], "b34b": p["b34b"], "gidn": p["gidn"][k],
        "wp": p["wp"], "bpb": p["bpb"],
    } for k in range(M)]
    r4 = _run(nc4, in4)
    out = np.concatenate([r4[k]["outk"] for k in range(M)], axis=0)
    return out.astype(np.float32)


# revision 7
# speedup vs baseline: 1.1085x; 1.1085x over previous
"""Trainium2 Bass kernel for nn_Classifier2_54022098649409 (GNN message passing).

Distribution (8 cores):
  L1: node-equal slices — dilated convs as shifted matmuls + per-node transform
      -> gather table T1 [N, 192] (cols 0:162 = h01@W11 | h02@W12 | h03@W13).
  L2/L3/L4: graph-boundary node slices (32 graphs/core); edges owned by dst core,
      grouped by 128-node dst block. Per edge chunk (128 edges):
      dma_gather rows from the table, build one-hot P'[e,r]=(iota==dstnorm)*ew on
      DVE, scatter-add via PE matmul P'^T @ msgs accumulated in PSUM per block.
  L2 -> ac_h1 -> T2 = ac_h1@W2 (pair-packed table [N/2, 64]).
  L3 -> ac_h2 (-> T3 pair-packed).
  L4 -> S=scatter(T3); ac_h3/4=relu(S@[W3|W4]+b); readout segment-sum by graph id
      via one-hot matmul, folded linear head W' = Wc1@Wc2.
Host does sharding/index prep + pure data-movement reassembly between launches.
"""

import numpy as np

import concourse.bacc as bacc
import concourse.bass as bass
import concourse.mybir as mybir
import concourse.tile as tile
from concourse.bass_utils import run_bass_kernel_spmd
from concourse.masks import make_identity

F32 = mybir.dt.float32
I16 = mybir.dt.int16
I32 = mybir.dt.int32
AOT = mybir.AluOpType

N, E, G, M = 51200, 819200, 256, 8
GPC = G // M          # graphs per core
SL = N // M           # L1 equal slice
HALF = N // 2         # 25600 (table half rows / pair rows)
F1, P1 = 162, 192     # L1 table width / padded width
FP = 64               # pair-packed row width (25 | pad | 25 | pad)

CORES = list(range(M))

# collected by test harness (exec_time_ns per launch when BASS_TRACE=1)
LAST_EXEC_NS = []

_prog_cache = {}


# --------------------------------------------------------------------------
# host-side prep
# --------------------------------------------------------------------------

def _host_prep(inputs):
    x = np.ascontiguousarray(inputs["x"], np.float32)
    src = np.asarray(inputs["src"]).astype(np.int64)
    dst = np.asarray(inputs["dst"]).astype(np.int64)
    gid = np.asarray(inputs["graph_ids"]).astype(np.int64)
    ew = np.ascontiguousarray(inputs["edge_weight"], np.float32)

    p = {}
    p["xT"] = np.ascontiguousarray(x.T)  # [128, N]

    # conv weights [128, 9*96]: block (d*3+k) = conv{d}_w[:, :, k].T
    cw = np.zeros((128, 9 * 96), np.float32)
    cb = np.zeros((96, 3), np.float32)
    for d in range(3):
        w = np.asarray(inputs[f"conv{d+1}_w"], np.float32)  # [96,128,3]
        for k in range(3):
            cw[:, (d * 3 + k) * 96:(d * 3 + k + 1) * 96] = w[:, :, k].T
        cb[:, d] = np.asarray(inputs[f"conv{d+1}_b"], np.float32)
    p["cw"], p["cb"] = cw, cb
    p["w1"] = np.ascontiguousarray(
        np.concatenate([np.asarray(inputs[w], np.float32)
                        for w in ("W11", "W12", "W13")], axis=1))  # [96,162]

    # graph-boundary slices
    starts = np.searchsorted(gid, np.arange(0, G, GPC))
    ends = np.concatenate([starts[1:], [N]])
    NB = int(np.max(np.ceil((ends - starts) / 128)).astype(int))
    p["starts"], p["ends"], p["NB"] = starts, ends, NB

    core_of_edge = np.searchsorted(starts, dst, side="right") - 1

    # ---- edge schedules ----
    def schedule(nhalf, half_arr, idx_arr):
        """Group edges per core by (dst block, half); pad runs to 128.
        Returns per-core dict of arrays + CPBH."""
        percore = []
        cnts = np.zeros((M, NB, nhalf), np.int64)
        for k in range(M):
            sel = np.where(core_of_edge == k)[0]
            dl = dst[sel] - starts[k]
            blk = dl // 128
            hh = half_arr[sel]
            order = np.lexsort((hh, blk))
            sel, dl, blk, hh = sel[order], dl[order], blk[order], hh[order]
            # run boundaries
            for b in range(NB):
                m0 = blk == b
                for h in range(nhalf):
                    cnts[k, b, h] = np.sum(m0 & (hh == h))
            percore.append((sel, dl))
        CPBH = int(np.max(np.ceil(cnts / 128)))
        out = []
        for k in range(M):
            sel, dl = percore[k]
            nch = NB * nhalf * CPBH
            idx = np.zeros(nch * 128, np.int16)
            dstn = np.zeros(nch * 128, np.float32)
            ewt = np.zeros(nch * 128, np.float32)
            par = np.zeros(nch * 128, np.float32)
            pos = 0
            run = 0
            for b in range(NB):
                for h in range(nhalf):
                    c = int(cnts[k, b, h])
                    s = slice(pos, pos + c)
                    o = run * 128 * CPBH
                    idx[o:o + c] = idx_arr[sel[s]]
                    dstn[o:o + c] = (dl[s] - b * 128).astype(np.float32)
                    ewt[o:o + c] = ew[sel[s]]
                    par[o:o + c] = ((src[sel[s]] % 128) // 64).astype(np.float32)
                    pos += c
                    run += 1
            out.append(dict(idx=idx, dstn=dstn, ewt=ewt, par=par))
        return out, CPBH

    sched2, CH2 = schedule(2, src // HALF, (src % HALF).astype(np.int16))
    pair = ((src // 128) * 64 + (src % 64)).astype(np.int16)
    sched3, CH3 = schedule(1, np.zeros(E, np.int64), pair)
    p["CH2"], p["CH3"] = CH2, CH3

    def finish(sched, CPBH, nhalf):
        out = []
        for k in range(M):
            s = sched[k]
            nch = NB * nhalf * CPBH
            d = dict(
                idx=np.ascontiguousarray(
                    np.tile(s["idx"].reshape(-1, 16).T, (8, 1))),  # [128, nch*8]
                dstn=np.ascontiguousarray(s["dstn"].reshape(nch, 128).T),  # [128, nch]
                ewt=np.ascontiguousarray(s["ewt"].reshape(nch, 128).T),
                par=np.ascontiguousarray(s["par"].reshape(nch, 128).T),
            )
            d["inv"] = np.ascontiguousarray(1.0 - d["par"])
            out.append(d)
        return out
    p["e2"] = finish(sched2, CH2, 2)
    p["e3"] = finish(sched3, CH3, 1)

    # small weights
    p["bia"] = np.ascontiguousarray(np.tile(np.concatenate(
        [np.asarray(inputs[b], np.float32) for b in ("b11", "b12", "b13")]
    )[None, :], (128, 1)))                                      # [128,162]
    p["w2"] = np.ascontiguousarray(np.asarray(inputs["W2"], np.float32))   # [54,25]
    p["b2b"] = np.ascontiguousarray(
        np.tile(np.asarray(inputs["b2"], np.float32)[None, :], (128, 1)))  # [128,25]
    p["w34"] = np.ascontiguousarray(np.concatenate(
        [np.asarray(inputs["W3"], np.float32),
         np.asarray(inputs["W4"], np.float32)], axis=1))        # [25,24]
    p["b34b"] = np.ascontiguousarray(np.tile(np.concatenate(
        [np.asarray(inputs["b3"], np.float32),
         np.asarray(inputs["b4"], np.float32)])[None, :], (128, 1)))  # [128,24]
    wp = (np.asarray(inputs["Wc1"], np.float32)
          @ np.asarray(inputs["Wc2"], np.float32)).astype(np.float32)  # [49,5]
    bp = (np.asarray(inputs["bc1"], np.float32)
          @ np.asarray(inputs["Wc2"], np.float32)
          + np.asarray(inputs["bc2"], np.float32)).astype(np.float32)  # [5]
    p["wp"] = np.ascontiguousarray(wp)
    p["bpb"] = np.ascontiguousarray(np.tile(bp[None, :], (GPC, 1)))  # [32,5]

    # per-core graph-id-norm per node tile [128, NB]
    gidn = []
    for k in range(M):
        g = np.full(NB * 128, -1.0, np.float32)
        n = ends[k] - starts[k]
        g[:n] = (gid[starts[k]:ends[k]] - k * GPC).astype(np.float32)
        gidn.append(np.ascontiguousarray(g.reshape(NB, 128).T))  # [128, NB]
    p["gidn"] = gidn
    return p


def _pack_pairs(tbl):
    """tbl [N, 25] -> [N/2, 64]: row b*64+j = [t[b*128+j] |pad| t[b*128+64+j] |pad]"""
    t = tbl.reshape(N // 128, 2, 64, 25)
    out = np.zeros((N // 2, FP), np.float32)
    o = out.reshape(N // 128, 64, FP)
    o[:, :, 0:25] = t[:, 0]
    o[:, :, 32:57] = t[:, 1]
    return out


# --------------------------------------------------------------------------
# bass program builders
# --------------------------------------------------------------------------

def _new_nc():
    return bacc.Bacc("TRN2", target_bir_lowering=False, debug=False,
                     enable_asserts=False, num_devices=M, num_swdge_queues=4)


def _build_l1():
    nc = _new_nc()
    xs = nc.dram_tensor("xs", [128, SL + 6], F32, kind="ExternalInput").ap()
    cw = nc.dram_tensor("cw", [128, 9 * 96], F32, kind="ExternalInput").ap()
    cb = nc.dram_tensor("cb", [96, 3], F32, kind="ExternalInput").ap()
    w1 = nc.dram_tensor("w1", [96, F1], F32, kind="ExternalInput").ap()
    t1 = nc.dram_tensor("t1", [SL, P1], F32, kind="ExternalOutput").ap()

    with tile.TileContext(nc) as tc:
        with (
            tc.tile_pool(name="consts", bufs=1) as consts,
            tc.tile_pool(name="work", bufs=3) as sb,
            tc.tile_pool(name="psc", bufs=4, space="PSUM") as ps_c,
            tc.tile_pool(name="pst", bufs=2, space="PSUM") as ps_t,
        ):
            xs_sb = consts.tile([128, SL + 6], F32)
            nc.sync.dma_start(out=xs_sb[:], in_=xs[:])
            cw_sb = consts.tile([128, 9 * 96], F32)
            nc.sync.dma_start(out=cw_sb[:], in_=cw[:])
            cb_sb = consts.tile([96, 3], F32)
            nc.sync.dma_start(out=cb_sb[:], in_=cb[:])
            w1_sb = consts.tile([96, F1], F32)
            nc.sync.dma_start(out=w1_sb[:], in_=w1[:])

            for t in range(SL // 128):
                c_sb = []
                for d in range(3):
                    pc = ps_c.tile([96, 128], F32, tag="pc")
                    for k in range(3):
                        off = 3 + t * 128 + (k - 1) * (d + 1)
                        nc.tensor.matmul(
                            out=pc[:],
                            lhsT=cw_sb[:, (d * 3 + k) * 96:(d * 3 + k + 1) * 96],
                            rhs=xs_sb[:, off:off + 128],
                            start=(k == 0), stop=(k == 2))
                    cs = sb.tile([96, 128], F32, tag=f"c{d}")
                    nc.vector.tensor_tensor(
                        out=cs[:], in0=pc[:],
                        in1=cb_sb[:, d:d + 1].to_broadcast([96, 128]),
                        op=AOT.add)
                    c_sb.append(cs)
                h_sb = []
                for i in range(3):
                    r = sb.tile([96, 128], F32, tag=f"h{i}")
                    nc.vector.tensor_scalar_max(out=r[:], in0=c_sb[i][:], scalar1=0.0)
                    nc.vector.tensor_tensor(
                        out=r[:], in0=r[:], in1=c_sb[(i + 1) % 3][:], op=AOT.add)
                    h_sb.append(r)
                o_sb = sb.tile([128, F1], F32, tag="o")
                for i in range(3):
                    pt = ps_t.tile([128, 54], F32, tag="pt")
                    nc.tensor.matmul(out=pt[:], lhsT=h_sb[i][:],
                                     rhs=w1_sb[:, i * 54:(i + 1) * 54],
                                     start=True, stop=True)
                    nc.vector.tensor_copy(out=o_sb[:, i * 54:(i + 1) * 54], in_=pt[:])
                nc.sync.dma_start(out=t1[t * 128:(t + 1) * 128, 0:F1], in_=o_sb[:])
    nc.compile()
    return nc


def _build_scatter_common(nc, tc, consts, NB, CH, nhalf):
    """Declare edge-schedule inputs + iota/identity and load them to SBUF."""
    nch = NB * nhalf * CH
    eidx = nc.dram_tensor("eidx", [128, nch * 8], I16, kind="ExternalInput").ap()
    edst = nc.dram_tensor("edst", [128, nch], F32, kind="ExternalInput").ap()
    eewt = nc.dram_tensor("eewt", [128, nch], F32, kind="ExternalInput").ap()

    eidx_sb = consts.tile([128, nch * 8], I16)
    nc.sync.dma_start(out=eidx_sb[:], in_=eidx[:])
    edst_sb = consts.tile([128, nch], F32)
    nc.sync.dma_start(out=edst_sb[:], in_=edst[:])
    eewt_sb = consts.tile([128, nch], F32)
    nc.sync.dma_start(out=eewt_sb[:], in_=eewt[:])

    iota_i = consts.tile([128, 128], I32)
    nc.gpsimd.iota(out=iota_i[:], pattern=[[1, 128]], base=0, channel_multiplier=0)
    iota_f = consts.tile([128, 128], F32)
    nc.vector.tensor_copy(out=iota_f[:], in_=iota_i[:])
    ident = consts.tile([128, 128], F32)
    make_identity(nc, ident[:])
    return eidx_sb, edst_sb, eewt_sb, iota_f, ident


def _build_l2(NB, CH):
    nc = _new_nc()
    t1lo = nc.dram_tensor("t1lo", [HALF, P1], F32, kind="ExternalInput").ap()
    t1hi = nc.dram_tensor("t1hi", [HALF, P1], F32, kind="ExternalInput").ap()
    bia = nc.dram_tensor("bia", [128, F1], F32, kind="ExternalInput").ap()
    w2 = nc.dram_tensor("w2", [54, 25], F32, kind="ExternalInput").ap()
    t2 = nc.dram_tensor("t2", [NB * 128, 25], F32, kind="ExternalOutput").ap()

    with tile.TileContext(nc) as tc:
        with (
            tc.tile_pool(name="consts", bufs=1) as consts,
            tc.tile_pool(name="gath", bufs=3) as gpool,
            tc.tile_pool(name="pm", bufs=4) as pmpool,
            tc.tile_pool(name="misc", bufs=3) as spool,
            tc.tile_pool(name="pss", bufs=2, space="PSUM") as ps_s,
            tc.tile_pool(name="pstr", bufs=2, space="PSUM") as ps_tr,
            tc.tile_pool(name="pso", bufs=2, space="PSUM") as ps_o,
        ):
            eidx_sb, edst_sb, eewt_sb, iota_f, ident = _build_scatter_common(
                nc, tc, consts, NB, CH, 2)
            bia_sb = consts.tile([128, F1], F32)
            nc.sync.dma_start(out=bia_sb[:], in_=bia[:])
            w2_sb = consts.tile([54, 25], F32)
            nc.sync.dma_start(out=w2_sb[:], in_=w2[:])

            for b in range(NB):
                ps = ps_s.tile([128, F1], F32, tag="s")
                for h in range(2):
                    call = b * 2 + h
                    g = gpool.tile([128, CH * P1], F32, tag="g")
                    nc.gpsimd.dma_gather(
                        out_ap=g[:].rearrange("p (c e) -> p c e", e=P1),
                        in_ap=(t1lo if h == 0 else t1hi)[:, :],
                        idxs_ap=eidx_sb[:, call * CH * 8:(call + 1) * CH * 8],
                        num_idxs=CH * 128,
                        num_idxs_reg=CH * 128,
                        elem_size=P1,
                        queue_num=call % 4,
                        single_packet=False,
                    )
                    for j in range(CH):
                        c = call * CH + j
                        pm = pmpool.tile([128, 128], F32, tag="pm")
                        nc.vector.tensor_tensor(
                            out=pm[:], in0=iota_f[:],
                            in1=edst_sb[:, c:c + 1].to_broadcast([128, 128]),
                            op=AOT.is_equal)
                        nc.vector.tensor_tensor(
                            out=pm[:], in0=pm[:],
                            in1=eewt_sb[:, c:c + 1].to_broadcast([128, 128]),
                            op=AOT.mult)
                        nc.tensor.matmul(
                            out=ps[:], lhsT=pm[:],
                            rhs=g[:, j * P1:j * P1 + F1],
                            start=(h == 0 and j == 0),
                            stop=(h == 1 and j == CH - 1))
                # ac_h1 = sum_i relu(S_i + b_i)
                tacc = spool.tile([128, F1], F32, tag="t")
                nc.vector.tensor_tensor(out=tacc[:], in0=ps[:], in1=bia_sb[:],
                                        op=AOT.add)
                nc.vector.tensor_scalar_max(out=tacc[:], in0=tacc[:], scalar1=0.0)
                ac = spool.tile([128, 54], F32, tag="ac")
                nc.vector.tensor_tensor(out=ac[:], in0=tacc[:, 0:54],
                                        in1=tacc[:, 54:108], op=AOT.add)
                nc.vector.tensor_tensor(out=ac[:], in0=ac[:],
                                        in1=tacc[:, 108:F1], op=AOT.add)
                ptr = ps_tr.tile([54, 128], F32, tag="tr")
                nc.tensor.transpose(out=ptr[:], in_=ac[:], identity=ident[:])
                acT = spool.tile([54, 128], F32, tag="acT")
                nc.vector.tensor_copy(out=acT[:], in_=ptr[:])
                po = ps_o.tile([128, 25], F32, tag="po")
                nc.tensor.matmul(out=po[:], lhsT=acT[:], rhs=w2_sb[:],
                                 start=True, stop=True)
                ot = spool.tile([128, 25], F32, tag="ot")
                nc.vector.tensor_copy(out=ot[:], in_=po[:])
                nc.sync.dma_start(out=t2[b * 128:(b + 1) * 128, :], in_=ot[:])
    nc.compile()
    return nc


def _scatter_pass_small(nc, tc, pools, b, CH, tbl, eidx_sb, edst_sb, eewt_sb,
                        epar_sb, einv_sb, iota_f):
    """Gather pair-packed rows + parity select + one-hot matmul scatter.
    Returns psum tile [128, 25] holding S for block b."""
    gpool, pmpool, mpool, ps_s = pools
    ps = ps_s.tile([128, 25], F32, tag="s")
    g = gpool.tile([128, CH * FP], F32, tag="g")
    nc.gpsimd.dma_gather(
        out_ap=g[:].rearrange("p (c e) -> p c e", e=FP),
        in_ap=tbl[:, :],
        idxs_ap=eidx_sb[:, b * CH * 8:(b + 1) * CH * 8],
        num_idxs=CH * 128,
        num_idxs_reg=CH * 128,
        elem_size=FP,
        queue_num=b % 4,
        single_packet=False,
    )
    for j in range(CH):
        c = b * CH + j
        m = mpool.tile([128, 25], F32, tag="m")
        nc.vector.tensor_tensor(
            out=m[:], in0=g[:, j * FP:j * FP + 25],
            in1=einv_sb[:, c:c + 1].to_broadcast([128, 25]), op=AOT.mult)
        m2 = mpool.tile([128, 25], F32, tag="m2")
        nc.vector.tensor_tensor(
            out=m2[:], in0=g[:, j * FP + 32:j * FP + 57],
            in1=epar_sb[:, c:c + 1].to_broadcast([128, 25]), op=AOT.mult)
        nc.vector.tensor_tensor(out=m[:], in0=m[:], in1=m2[:], op=AOT.add)
        pm = pmpool.tile([128, 128], F32, tag="pm")
        nc.vector.tensor_tensor(
            out=pm[:], in0=iota_f[:],
            in1=edst_sb[:, c:c + 1].to_broadcast([128, 128]), op=AOT.is_equal)
        nc.vector.tensor_tensor(
            out=pm[:], in0=pm[:],
            in1=eewt_sb[:, c:c + 1].to_broadcast([128, 128]), op=AOT.mult)
        nc.tensor.matmul(out=ps[:], lhsT=pm[:], rhs=m[:],
                         start=(j == 0), stop=(j == CH - 1))
    return ps


def _declare_parity(nc, consts, NB, CH):
    nch = NB * CH
    epar = nc.dram_tensor("epar", [128, nch], F32, kind="ExternalInput").ap()
    einv = nc.dram_tensor("einv", [128, nch], F32, kind="ExternalInput").ap()
    epar_sb = consts.tile([128, nch], F32)
    nc.sync.dma_start(out=epar_sb[:], in_=epar[:])
    einv_sb = consts.tile([128, nch], F32)
    nc.sync.dma_start(out=einv_sb[:], in_=einv[:])
    return epar_sb, einv_sb


def _build_l3(NB, CH):
    nc = _new_nc()
    t2p = nc.dram_tensor("t2p", [HALF, FP], F32, kind="ExternalInput").ap()
    b2b = nc.dram_tensor("b2b", [128, 25], F32, kind="ExternalInput").ap()
    ach2 = nc.dram_tensor("ach2", [NB * 128, 25], F32, kind="ExternalOutput").ap()

    with tile.TileContext(nc) as tc:
        with (
            tc.tile_pool(name="consts", bufs=1) as consts,
            tc.tile_pool(name="gath", bufs=3) as gpool,
            tc.tile_pool(name="pm", bufs=4) as pmpool,
            tc.tile_pool(name="msel", bufs=4) as mpool,
            tc.tile_pool(name="misc", bufs=3) as spool,
            tc.tile_pool(name="pss", bufs=2, space="PSUM") as ps_s,
        ):
            eidx_sb, edst_sb, eewt_sb, iota_f, ident = _build_scatter_common(
                nc, tc, consts, NB, CH, 1)
            epar_sb, einv_sb = _declare_parity(nc, consts, NB, CH)
            b2b_sb = consts.tile([128, 25], F32)
            nc.sync.dma_start(out=b2b_sb[:], in_=b2b[:])

            pools = (gpool, pmpool, mpool, ps_s)
            for b in range(NB):
                ps = _scatter_pass_small(nc, tc, pools, b, CH, t2p, eidx_sb,
                                         edst_sb, eewt_sb, epar_sb, einv_sb,
                                         iota_f)
                ac = spool.tile([128, 25], F32, tag="ac")
                nc.vector.tensor_tensor(out=ac[:], in0=ps[:], in1=b2b_sb[:],
                                        op=AOT.add)
                nc.vector.tensor_scalar_max(out=ac[:], in0=ac[:], scalar1=0.0)
                nc.sync.dma_start(out=ach2[b * 128:(b + 1) * 128, :], in_=ac[:])
    nc.compile()
    return nc


def _build_l4(NB, CH):
    nc = _new_nc()
    t3p = nc.dram_tensor("t3p", [HALF, FP], F32, kind="ExternalInput").ap()
    ach2k = nc.dram_tensor("ach2k", [NB * 128, 25], F32, kind="ExternalInput").ap()
    w34 = nc.dram_tensor("w34", [25, 24], F32, kind="ExternalInput").ap()
    b34b = nc.dram_tensor("b34b", [128, 24], F32, kind="ExternalInput").ap()
    gidn = nc.dram_tensor("gidn", [128, NB], F32, kind="ExternalInput").ap()
    wp = nc.dram_tensor("wp", [49, 5], F32, kind="ExternalInput").ap()
    bpb = nc.dram_tensor("bpb", [GPC, 5], F32, kind="ExternalInput").ap()
    outk = nc.dram_tensor("outk", [GPC, 5], F32, kind="ExternalOutput").ap()

    with tile.TileContext(nc) as tc:
        with (
            tc.tile_pool(name="consts", bufs=1) as consts,
            tc.tile_pool(name="gath", bufs=3) as gpool,
            tc.tile_pool(name="pm", bufs=4) as pmpool,
            tc.tile_pool(name="msel", bufs=4) as mpool,
            tc.tile_pool(name="misc", bufs=3) as spool,
            tc.tile_pool(name="pss", bufs=2, space="PSUM") as ps_s,
            tc.tile_pool(name="pstr", bufs=2, space="PSUM") as ps_tr,
            tc.tile_pool(name="psh", bufs=2, space="PSUM") as ps_h,
            tc.tile_pool(name="psr", bufs=1, space="PSUM") as ps_r,
            # PSUM budget: pss 2 + pstr 2 (shared tag) + psh 2 (shared tag)
            # + psr 1 = 7 banks
        ):
            eidx_sb, edst_sb, eewt_sb, iota_f, ident = _build_scatter_common(
                nc, tc, consts, NB, CH, 1)
            epar_sb, einv_sb = _declare_parity(nc, consts, NB, CH)
            w34_sb = consts.tile([25, 24], F32)
            nc.sync.dma_start(out=w34_sb[:], in_=w34[:])
            b34b_sb = consts.tile([128, 24], F32)
            nc.sync.dma_start(out=b34b_sb[:], in_=b34b[:])
            gidn_sb = consts.tile([128, NB], F32)
            nc.sync.dma_start(out=gidn_sb[:], in_=gidn[:])
            wp_sb = consts.tile([49, 5], F32)
            nc.sync.dma_start(out=wp_sb[:], in_=wp[:])
            bpb_sb = consts.tile([GPC, 5], F32)
            nc.sync.dma_start(out=bpb_sb[:], in_=bpb[:])

            pr = ps_r.tile([GPC, 49], F32, tag="r")
            pools = (gpool, pmpool, mpool, ps_s)
            for b in range(NB):
                ps = _scatter_pass_small(nc, tc, pools, b, CH, t3p, eidx_sb,
                                         edst_sb, eewt_sb, epar_sb, einv_sb,
                                         iota_f)
                s_sb = spool.tile([128, 25], F32, tag="sb")
                nc.vector.tensor_copy(out=s_sb[:], in_=ps[:])
                ptr = ps_tr.tile([25, 128], F32, tag="tr")
                nc.tensor.transpose(out=ptr[:], in_=s_sb[:], identity=ident[:])
                st = spool.tile([25, 128], F32, tag="st")
                nc.vector.tensor_copy(out=st[:], in_=ptr[:])
                ph = ps_h.tile([128, 24], F32, tag="h")
                nc.tensor.matmul(out=ph[:], lhsT=st[:], rhs=w34_sb[:],
                                 start=True, stop=True)
                feat = spool.tile([128, 49], F32, tag="feat")
                nc.sync.dma_start(out=feat[:, 0:25],
                                  in_=ach2k[b * 128:(b + 1) * 128, :])
                nc.vector.tensor_tensor(out=feat[:, 25:49], in0=ph[:],
                                        in1=b34b_sb[:], op=AOT.add)
                nc.vector.tensor_scalar_max(out=feat[:, 25:49],
                                            in0=feat[:, 25:49], scalar1=0.0)
                pg = pmpool.tile([128, GPC], F32, tag="pg")
                nc.vector.tensor_tensor(
                    out=pg[:], in0=iota_f[:, 0:GPC],
                    in1=gidn_sb[:, b:b + 1].to_broadcast([128, GPC]),
                    op=AOT.is_equal)
                nc.tensor.matmul(out=pr[:], lhsT=pg[:], rhs=feat[:],
                                 start=(b == 0), stop=(b == NB - 1))
            r_sb = spool.tile([GPC, 49], F32, tag="rsb")
            nc.vector.tensor_copy(out=r_sb[:], in_=pr[:])
            prt = ps_tr.tile([49, GPC], F32, tag="tr")
            nc.tensor.transpose(out=prt[:], in_=r_sb[:],
                                identity=ident[0:GPC, 0:GPC])
            rt_sb = spool.tile([49, GPC], F32, tag="rts")
            nc.vector.tensor_copy(out=rt_sb[:], in_=prt[:])
            po = ps_h.tile([GPC, 5], F32, tag="h")
            nc.tensor.matmul(out=po[:], lhsT=rt_sb[:], rhs=wp_sb[:],
                             start=True, stop=True)
            o_sb = spool.tile([GPC, 5], F32, tag="ob")
            nc.vector.tensor_tensor(out=o_sb[:], in0=po[:], in1=bpb_sb[:],
                                    op=AOT.add)
            nc.sync.dma_start(out=outk[:], in_=o_sb[:])
    nc.compile()
    return nc


# --------------------------------------------------------------------------
# run helpers
# --------------------------------------------------------------------------

def _run(nc, in_maps):
    res = run_bass_kernel_spmd(nc, in_maps, core_ids=CORES)
    if res.exec_time_ns is not None:
        LAST_EXEC_NS.append(res.exec_time_ns)
    return res.results


def _get_prog(key, builder, *args):
    if key not in _prog_cache:
        _prog_cache[key] = builder(*args)
    return _prog_cache[key]


def kernel(**inputs):
    LAST_EXEC_NS.clear()
    p = _host_prep(inputs)
    NB, CH2, CH3 = p["NB"], p["CH2"], p["CH3"]
    starts, ends = p["starts"], p["ends"]

    # ---------------- L1 ----------------
    nc1 = _get_prog("l1", _build_l1)
    xTp = np.zeros((128, N + 6), np.float32)
    xTp[:, 3:3 + N] = p["xT"]
    in1 = [{
        "xs": np.ascontiguousarray(xTp[:, k * SL:k * SL + SL + 6]),
        "cw": p["cw"], "cb": p["cb"], "w1": p["w1"],
    } for k in range(M)]
    r1 = _run(nc1, in1)
    T1 = np.concatenate([r1[k]["t1"] for k in range(M)], axis=0)
    T1[:, F1:] = 0.0
    t1lo = np.ascontiguousarray(T1[:HALF])
    t1hi = np.ascontiguousarray(T1[HALF:])

    # ---------------- L2 ----------------
    nc2 = _get_prog(("l2", NB, CH2), _build_l2, NB, CH2)
    in2 = [{
        "t1lo": t1lo, "t1hi": t1hi,
        "eidx": p["e2"][k]["idx"], "edst": p["e2"][k]["dstn"],
        "eewt": p["e2"][k]["ewt"],
        "bia": p["bia"], "w2": p["w2"],
    } for k in range(M)]
    r2 = _run(nc2, in2)
    T2g = np.zeros((N, 25), np.float32)
    for k in range(M):
        n = ends[k] - starts[k]
        T2g[starts[k]:ends[k]] = r2[k]["t2"][:n]
    t2p = _pack_pairs(T2g)

    # ---------------- L3 ----------------
    nc3 = _get_prog(("l3", NB, CH3), _build_l3, NB, CH3)
    in3 = [{
        "t2p": t2p,
        "eidx": p["e3"][k]["idx"], "edst": p["e3"][k]["dstn"],
        "eewt": p["e3"][k]["ewt"], "epar": p["e3"][k]["par"],
        "einv": p["e3"][k]["inv"],
        "b2b": p["b2b"],
    } for k in range(M)]
    r3 = _run(nc3, in3)
    ach2 = [r3[k]["ach2"] for k in range(M)]
    T3g = np.zeros((N, 25), np.float32)
    for k in range(M):
        n = ends[k] - starts[k]
        T3g[starts[k]:ends[k]] = ach2[k][:n]
    t3p = _pack_pairs(T3g)

    # ---------------- L4 ----------------
    nc4 = _get_prog(("l4", NB, CH3), _build_l4, NB, CH3)
    in4 = [{
        "t3p": t3p,
        "eidx": p["e3"][k]["idx"], "edst": p["e3"][k]["dstn"],
        "eewt": p["e3"][k]["ewt"], "epar": p["e3"][k]["par"],
        "einv": p["e3"][k]["inv"],
        "ach2k": np.ascontiguousarray(ach2[k]),
        "w34": p["w34"], "b34b": p["b34b"], "gidn": p["gidn"][k],
        "wp": p["wp"], "bpb": p["bpb"],
    } for k in range(M)]
    r4 = _run(nc4, in4)
    out = np.concatenate([r4[k]["outk"] for k in range(M)], axis=0)
    return out.astype(np.float32)
